# revision 1
# baseline (speedup 1.0000x reference)
"""Trainium2 Bass kernel for a pre-LN transformer block (B=4, T=2048, C=512, H=8).

Sharding: 8 cores, 2 per batch element. Each core handles 4 q-chunks of 256
tokens (core group g takes chunks {2i+g}), with causal k-extents padded to a
uniform schedule {512, 1024, 1536, 2048} so all cores run the same SPMD
program; padding + the causal diagonal are neutralized by multiplicative
{0,1} masks fed as per-core data (last 4 k-tiles of each slot).

On-device math:
  LN stats via ones-matmuls (mean) and squared ones-matmuls (var); rstd via
  exp(-0.5*ln(var+eps)). The LN mean-subtraction is folded into QKV / FF1
  projections as a K=1 rank-1 correction matmul (-colsum(W) (x) mu); the
  per-token rstd is fused into PSUM-evacuation multiplies. g1/g2 are folded
  into the weights host-side; all biases are zero (asserted).
Attention: S^T = K_h^T q with 2 heads packed per 128-row PE pass, exp on ACT
  over 2-bank PSUM groups, masks, then AV via token-major V augmented with a
  ones column so the softmax denominator falls out of the same matmul (M=65).
Attention runs in two sequential head-pair groups to halve K/V/Q residency.
"""

import os
import sys

sys.path.insert(0, "/opt/trn_rl_repo")

import contextlib

import numpy as np

import concourse.bass as bass
import concourse.tile as tile
from concourse import bacc, mybir
from concourse.bass_utils import run_bass_kernel_spmd

P = 128
C = 512
T = 2048
TQ = 1024
H = 8
HS = 64
F = 2048
NS = 4            # c-subtiles of C
NSLOT = 4         # q-chunks (slots) per core, 256 tokens each
QC = 256          # q-chunk width
EXTS = [512, 1024, 1536, 2048]   # scheduled k-extent per slot
EPS = 1e-5

f32 = mybir.dt.float32
f32r = mybir.dt.float32r
AF = mybir.ActivationFunctionType
ALU = mybir.AluOpType

_last_exec_time_ns = None
_last_results = None


def _build_program(limit="full"):
    nc = bacc.Bacc(name="block")

    def inp(name, shape):
        return nc.declare_dram_parameter(name, list(shape), f32, isOutput=False)

    xkT = inp("xkT", (C, T))          # x[b].T
    xqT = inp("xqT", (C, TQ))         # q-rows of x[b], transposed, slot order
    wqT = inp("wqT", (C, C))          # (Wq*g1).T * C^-0.5
    wkT = inp("wkT", (C, C))
    wvT = inp("wvT", (C, C))
    woT = inp("woT", (C, C))
    w1T = inp("w1T", (C, F))          # (W_ff1*g2).T
    w2T = inp("w2T", (F, C))
    nwqcs = inp("nwqcs", (1, C))      # -colsum(wqT)
    nwkcs = inp("nwkcs", (1, C))
    nwvcs = inp("nwvcs", (1, C))
    nw1cs = inp("nw1cs", (1, F))
    wocs = inp("wocs", (P, NS))       # colsum_j Wo[j, c'] as column tiles
    masks = inp("masks", (P, NSLOT, 4, QC))  # last-4 kt masks per slot
    cst = inp("cst", (P, P))          # ones
    yT = nc.declare_dram_parameter("yT", [C, TQ], f32, isOutput=True)
    scr = nc.dram_tensor("scratch_rk", [1, T], f32)

    def _body(tc, top):
        # ---------- whole-kernel persistent pool (small) ----------
        pc = top.enter_context(tc.tile_pool(name="const", bufs=1))
        ones_r = pc.tile([P, 1], f32r, tag="ones_r")
        nc.sync.dma_start(out=ones_r, in_=cst.ap()[:, 0:1].bitcast(f32r))
        eps_sb = pc.tile([1, 1], f32, tag="eps")
        nc.vector.memset(eps_sb, EPS)
        wocs_sb = pc.tile([P, NS], f32r, tag="wocs")
        nc.sync.dma_start(out=wocs_sb, in_=wocs.ap().bitcast(f32r))

        # ---------- spans A..C: xqT; spans B..C: attnT ----------
        pAC = top.enter_context(tc.tile_pool(name="pAC", bufs=1))
        xqT_sb = pAC.tile([P, NS, TQ], f32r, tag="xqT")          # 16KB
        for s in range(NS):
            nc.sync.dma_start(out=xqT_sb[:, s], in_=xqT.ap()[s * P:(s + 1) * P, :].bitcast(f32r))
        attnT_sb = pAC.tile([P, NS, TQ], f32r, tag="attnT")      # 16KB

        # ============ stats + per-group (projections + attention) ============
        with tc.tile_pool(name="pStats", bufs=1) as pst_sb, \
             tc.tile_pool(name="pW", bufs=1) as pw, \
             tc.tile_pool(name="pStream", bufs=3) as pstr, \
             tc.tile_pool(name="pX2", bufs=2) as px2:

            # QKV weights (24KB)
            wq_sb = pw.tile([P, NS, C], f32r, tag="wq")
            wk_sb = pw.tile([P, NS, C], f32r, tag="wk")
            wv_sb = pw.tile([P, NS, C], f32r, tag="wv")
            for s in range(NS):
                nc.sync.dma_start(out=wq_sb[:, s], in_=wqT.ap()[s * P:(s + 1) * P, :].bitcast(f32r))
                nc.sync.dma_start(out=wk_sb[:, s], in_=wkT.ap()[s * P:(s + 1) * P, :].bitcast(f32r))
                nc.sync.dma_start(out=wv_sb[:, s], in_=wvT.ap()[s * P:(s + 1) * P, :].bitcast(f32r))
            # masks (16KB), span both groups
            mask_sb = pw.tile([P, NSLOT, 4, QC], f32r, tag="masks")
            for sl_ in range(NSLOT):
                nc.sync.dma_start(out=mask_sb[:, sl_], in_=masks.ap()[:, sl_].bitcast(f32r))

            # stats rows (span both groups)
            muk_row = pst_sb.tile([1, T], f32r, tag="muk")       # 8KB
            rstdk_row = pst_sb.tile([1, T], f32, tag="rstdk")    # 8KB (also e2 dest)
            tmp_row = pst_sb.tile([1, T], f32, tag="tmprow")     # 8KB
            muq_row = pst_sb.tile([1, TQ], f32r, tag="muq")      # 4KB
            rstdq_row = pst_sb.tile([1, TQ], f32, tag="rstdq")   # 4KB
            nwqcs_sb = pst_sb.tile([1, C], f32r, tag="nwqcs")
            nc.sync.dma_start(out=nwqcs_sb, in_=nwqcs.ap().bitcast(f32r))
            nwkcs_sb = pst_sb.tile([1, C], f32r, tag="nwkcs")
            nc.sync.dma_start(out=nwkcs_sb, in_=nwkcs.ap().bitcast(f32r))
            nwvcs_sb = pst_sb.tile([1, C], f32r, tag="nwvcs")
            nc.sync.dma_start(out=nwvcs_sb, in_=nwvcs.ap().bitcast(f32r))
            rkb_sb = pst_sb.tile([P, T], f32, tag="rkb")         # 8KB
            rqb_sb = pst_sb.tile([P, TQ], f32, tag="rqb")        # 4KB
            rstdk_col = pst_sb.tile([P, T // P], f32, tag="rkcol")

            # ---- LN1 stats (xk streamed, xq resident) ----
            with tc.tile_pool(name="pStPs", bufs=4, space="PSUM") as pstp:
                for tch in range(T // 512):
                    sl = slice(tch * 512, (tch + 1) * 512)
                    ps_x = pstp.tile([1, 512], f32, tag="st_x", name=f"stxk{tch}")
                    ps_2 = pstp.tile([1, 512], f32, tag="st_2", name=f"st2k{tch}")
                    for s in range(NS):
                        xs = pstr.tile([P, 512], f32r, tag="xks", name=f"xks{tch}_{s}")
                        nc.sync.dma_start(out=xs, in_=xkT.ap()[s * P:(s + 1) * P, sl].bitcast(f32r))
                        nc.tensor.matmul(ps_x, ones_r, xs, start=(s == 0), stop=(s == NS - 1))
                        x2 = px2.tile([P, 512], f32r, tag="x2", name=f"x2k{tch}_{s}")
                        nc.vector.tensor_tensor(out=x2, in0=xs, in1=xs, op=ALU.mult)
                        nc.tensor.matmul(ps_2, ones_r, x2, start=(s == 0), stop=(s == NS - 1))
                    nc.vector.tensor_scalar_mul(out=muk_row[:, sl], in0=ps_x, scalar1=1.0 / C)
                    nc.vector.tensor_scalar_mul(out=rstdk_row[:, sl], in0=ps_2, scalar1=1.0 / C)
                for tch in range(TQ // 512):
                    sl = slice(tch * 512, (tch + 1) * 512)
                    ps_x = pstp.tile([1, 512], f32, tag="st_x", name=f"stxq{tch}")
                    ps_2 = pstp.tile([1, 512], f32, tag="st_2", name=f"st2q{tch}")
                    for s in range(NS):
                        nc.tensor.matmul(ps_x, ones_r, xqT_sb[:, s, sl],
                                         start=(s == 0), stop=(s == NS - 1))
                        x2 = px2.tile([P, 512], f32r, tag="x2", name=f"x2q{tch}_{s}")
                        nc.vector.tensor_tensor(out=x2, in0=xqT_sb[:, s, sl],
                                                in1=xqT_sb[:, s, sl], op=ALU.mult)
                        nc.tensor.matmul(ps_2, ones_r, x2, start=(s == 0), stop=(s == NS - 1))
                    nc.vector.tensor_scalar_mul(out=muq_row[:, sl], in0=ps_x, scalar1=1.0 / C)
                    nc.vector.tensor_scalar_mul(out=rstdq_row[:, sl], in0=ps_2, scalar1=1.0 / C)

            def finish_rstd(mu_row, rstd_row, t_row):
                # rstd <- exp(-0.5*ln((E[x^2] - mu^2) + eps)); rstd_row holds E[x^2]
                nc.vector.tensor_tensor(out=t_row, in0=mu_row, in1=mu_row, op=ALU.mult)
                nc.vector.tensor_tensor(out=rstd_row, in0=rstd_row, in1=t_row, op=ALU.subtract)
                nc.scalar.activation(out=rstd_row, in_=rstd_row, func=AF.Ln, bias=eps_sb)
                nc.scalar.activation(out=rstd_row, in_=rstd_row, func=AF.Exp, scale=-0.5)

            finish_rstd(muk_row, rstdk_row, tmp_row)
            finish_rstd(muq_row, rstdq_row, tmp_row[:, 0:TQ])
            nc.gpsimd.partition_broadcast(rkb_sb, rstdk_row)
            nc.gpsimd.partition_broadcast(rqb_sb, rstdq_row)
            # partition-scatter sbuf->sbuf DMA corrupts data on HW; round-trip
            # through DRAM, whose DMA distributes across partitions correctly
            nc.sync.dma_start(out=scr.ap(), in_=rstdk_row)
            nc.sync.dma_start(out=rstdk_col, in_=scr.ap().rearrange("a (o p) -> (a p) o", p=P))
            if limit == "stats":
                nc.sync.dma_start(out=yT.ap()[0:1, 0:TQ], in_=rstdk_row[:, 0:TQ])
                nc.sync.dma_start(out=yT.ap()[1:2, 0:TQ], in_=rstdk_row[:, TQ:T])
                nc.sync.dma_start(out=yT.ap()[2:3, 0:TQ], in_=muk_row[:, 0:TQ].bitcast(f32))
                nc.sync.dma_start(out=yT.ap()[3:4, 0:TQ], in_=muk_row[:, TQ:T].bitcast(f32))
                nc.sync.dma_start(out=yT.ap()[4:4 + P, 0:T // P], in_=rstdk_col)
                return

            # ---- two head-pair groups: projections then attention ----
            for grp in range(2 if limit not in ("proj1", "attn1") else 1):
                with tc.tile_pool(name=f"pG{grp}", bufs=1) as pg:
                    kT_sb = pg.tile([P, 2, T], f32r, tag="kT")           # 16KB
                    vaug_sb = pg.tile([P, T // P, 4 * 65], f32r, tag="vaug")  # 16.6KB
                    qT_sb = pg.tile([P, 2, TQ], f32r, tag="qT")          # 8KB
                    nc.sync.dma_start(
                        out=vaug_sb.rearrange("p t (h x) -> p t h x", x=65)[:, :, :, 64:65],
                        in_=cst.ap()[:, 0:64].rearrange("p (t h x) -> p t h x", h=4, x=1).bitcast(f32r),
                    )

                    with tc.tile_pool(name=f"pPrj{grp}", bufs=3, space="PSUM") as pap:
                        # K projection for this group's 2 feature tiles
                        for jj in range(2):
                            j = 2 * grp + jj
                            for tch in range(T // 512):
                                sl = slice(tch * 512, (tch + 1) * 512)
                                ps = pap.tile([P, 512], f32, tag="proj", name=f"k{grp}_{jj}_{tch}")
                                for s in range(NS):
                                    xs = pstr.tile([P, 512], f32r, tag="xks", name=f"kx{grp}_{jj}_{tch}_{s}")
                                    nc.sync.dma_start(out=xs, in_=xkT.ap()[s * P:(s + 1) * P, sl].bitcast(f32r))
                                    nc.tensor.matmul(ps, wk_sb[:, s, j * P:(j + 1) * P], xs,
                                                     start=(s == 0), stop=False)
                                nc.tensor.matmul(ps, nwkcs_sb[:, j * P:(j + 1) * P], muk_row[:, sl],
                                                 start=False, stop=True)
                                nc.vector.tensor_tensor(out=kT_sb[:, jj, sl], in0=ps,
                                                        in1=rkb_sb[:, sl], op=ALU.mult)
                        # V projection (token-major into V_aug)
                        for tt in range(T // P):
                            tsl = slice(tt * P, (tt + 1) * P)
                            ps = pap.tile([P, 256], f32, tag="projv", name=f"v{grp}_{tt}")
                            xs = pstr.tile([P, NS, P], f32r, tag="xkv", name=f"vx{grp}_{tt}")
                            for s2 in range(NS):
                                nc.sync.dma_start(
                                    out=xs[:, s2],
                                    in_=xkT.ap()[s2 * P:(s2 + 1) * P, tsl].bitcast(f32r))
                            for s in range(NS):
                                nc.tensor.matmul(ps, xs[:, s], wv_sb[:, s, 256 * grp:256 * (grp + 1)],
                                                 start=(s == 0), stop=False)
                            nc.tensor.matmul(ps, muk_row[:, tsl], nwvcs_sb[:, 256 * grp:256 * (grp + 1)],
                                             start=False, stop=True)
                            nc.vector.tensor_scalar_mul(
                                out=vaug_sb[:, tt].rearrange("p (h x) -> p h x", x=65)[:, :, 0:64],
                                in0=ps.rearrange("p (h d) -> p h d", d=HS),
                                scalar1=rstdk_col[:, tt:tt + 1])
                        # Q projection
                        for jj in range(2):
                            j = 2 * grp + jj
                            for tch in range(TQ // 512):
                                sl = slice(tch * 512, (tch + 1) * 512)
                                ps = pap.tile([P, 512], f32, tag="proj", name=f"q{grp}_{jj}_{tch}")
                                for s in range(NS):
                                    nc.tensor.matmul(ps, wq_sb[:, s, j * P:(j + 1) * P],
                                                     xqT_sb[:, s, sl], start=(s == 0), stop=False)
                                nc.tensor.matmul(ps, nwqcs_sb[:, j * P:(j + 1) * P], muq_row[:, sl],
                                                 start=False, stop=True)
                                nc.vector.tensor_tensor(out=qT_sb[:, jj, sl], in0=ps,
                                                        in1=rqb_sb[:, sl], op=ALU.mult)

                    if limit in ("proj1", "proj"):
                        nc.sync.dma_start(out=yT.ap()[0:P, 0:TQ], in_=kT_sb[:, 0, 0:TQ].bitcast(f32))
                        continue
                    # ---- attention for this group's 2 head-pairs ----
                    with tc.tile_pool(name=f"pP{grp}", bufs=3) as pp, \
                         tc.tile_pool(name=f"pEps{grp}", bufs=3) as pe, \
                         tc.tile_pool(name=f"pSps{grp}", bufs=2, space="PSUM") as pbp, \
                         tc.tile_pool(name=f"pAVps{grp}", bufs=2, space="PSUM") as pbo:
                        for jj in range(2):
                            hp = 2 * grp + jj
                            for slot in range(NSLOT):
                                nkt = EXTS[slot] // P
                                qsl = slice(slot * QC, (slot + 1) * QC)
                                # one accumulator bank per head: interleaved
                                # accumulation groups must not share a bank
                                # (start=True clears the whole bank's
                                # has_written bits)
                                po = [pbo.tile([65, QC], f32, tag=f"av{hi}",
                                               name=f"av{hp}_{slot}_{hi}")
                                      for hi in range(2)]
                                pending = None

                                def emit_av(ktp, p_tile, po=po, nkt=nkt):
                                    for i in range(2):
                                        kt = 2 * ktp + i
                                        for hi in range(2):
                                            h_loc = 2 * jj + hi
                                            nc.tensor.matmul(
                                                po[hi],
                                                vaug_sb[:, kt, h_loc * 65:(h_loc + 1) * 65],
                                                p_tile[:, hi, i, :],
                                                start=(kt == 0),
                                                stop=(kt == nkt - 1),
                                            )

                                for ktp in range(nkt // 2):
                                    # psum layout [P, head, kt-parity, QC]: each
                                    # bank hosts a single PE row-group — base-0
                                    # and base-64 matmuls sharing a bank return
                                    # garbage on HW at scale
                                    sp = pbp.tile([P, 2, 2, QC], f32, tag="spair",
                                                  name=f"s{hp}_{slot}_{ktp}")
                                    for i in range(2):
                                        kt = 2 * ktp + i
                                        ksl = slice(kt * P, (kt + 1) * P)
                                        nc.tensor.matmul(sp[:, 0, i, :], kT_sb[0:64, jj, ksl],
                                                         qT_sb[0:64, jj, qsl], start=True, stop=True)
                                        nc.tensor.matmul(sp[:, 1, i, :], kT_sb[64:128, jj, ksl],
                                                         qT_sb[64:128, jj, qsl], start=True, stop=True)
                                    pt = pp.tile([P, 2, 2, QC], f32r, tag="p",
                                                 name=f"p{hp}_{slot}_{ktp}")
                                    nc.scalar.activation(out=pt, in_=sp, func=AF.Exp)
                                    for i in range(2):
                                        kt = 2 * ktp + i
                                        if kt >= nkt - 4:
                                            eng = nc.vector if (kt % 2 == 0) else nc.gpsimd
                                            m = mask_sb[:, slot, kt - (nkt - 4)]
                                            for hi in range(2):
                                                eng.tensor_tensor(
                                                    out=pt[:, hi, i, :],
                                                    in0=pt[:, hi, i, :],
                                                    in1=m, op=ALU.mult)
                                    if pending is not None:
                                        emit_av(*pending)
                                    pending = (ktp, pt)
                                emit_av(*pending)

                                for hi in range(2):
                                    r_row = pe.tile([1, QC], f32, tag="r",
                                                    name=f"r{hp}_{slot}_{hi}")
                                    nc.vector.reciprocal(out=r_row, in_=po[hi][64:65, :])
                                    rrep = pe.tile([64, QC], f32, tag="rrep",
                                                   name=f"rr{hp}_{slot}_{hi}")
                                    nc.gpsimd.partition_broadcast(rrep, r_row)
                                    nc.vector.tensor_tensor(
                                        out=attnT_sb[hi * 64:(hi + 1) * 64, hp, qsl],
                                        in0=po[hi][0:64, :], in1=rrep, op=ALU.mult)

        if limit in ("proj1", "proj", "attn1", "attn"):
            if limit in ("attn1", "attn"):
                for s in range(NS):
                    nc.sync.dma_start(out=yT.ap()[s * P:(s + 1) * P, :], in_=attnT_sb[:, s].bitcast(f32))
            return

        # ================= Phase C: Wo + residual + LN2 stats =================
        with tc.tile_pool(name="pCD", bufs=1) as pcd, \
             tc.tile_pool(name="pCDrows", bufs=1) as pcr:
            wo_sb = pcd.tile([P, NS, C], f32r, tag="wo")
            for s in range(NS):
                nc.sync.dma_start(out=wo_sb[:, s], in_=woT.ap()[s * P:(s + 1) * P, :].bitcast(f32r))
            w1_sb = pcd.tile([P, NS, F], f32r, tag="w1")
            for s in range(NS):
                nc.sync.dma_start(out=w1_sb[:, s], in_=w1T.ap()[s * P:(s + 1) * P, :].bitcast(f32r))
            w2_sb = pcd.tile([P, F // P, C], f32r, tag="w2")
            for s in range(F // P):
                nc.sync.dma_start(out=w2_sb[:, s], in_=w2T.ap()[s * P:(s + 1) * P, :].bitcast(f32r))
            xnewT_sb = pcd.tile([P, NS, TQ], f32, tag="xnewT")
            xnewTr_sb = pcd.tile([P, NS, TQ], f32r, tag="xnewTr")
            mu2_row = pcr.tile([1, TQ], f32r, tag="mu2")
            rstd2_row = pcr.tile([1, TQ], f32, tag="rstd2")
            t2_row = pcr.tile([1, TQ], f32, tag="t2row")
            nw1cs_sb = pcr.tile([1, F], f32r, tag="nw1cs")
            nc.sync.dma_start(out=nw1cs_sb, in_=nw1cs.ap().bitcast(f32r))
            r2b_sb = pcr.tile([P, TQ], f32, tag="r2b")

            with tc.tile_pool(name="pC2", bufs=2) as pcc, \
                 tc.tile_pool(name="pCps", bufs=3, space="PSUM") as pcp, \
                 tc.tile_pool(name="pCst", bufs=2, space="PSUM") as pcs:
                for j in range(NS):
                    for tch in range(TQ // 512):
                        sl = slice(tch * 512, (tch + 1) * 512)
                        ps = pcp.tile([P, 512], f32, tag="proj", name=f"wo{j}_{tch}")
                        for s in range(NS):
                            nc.tensor.matmul(ps, wo_sb[:, s, j * P:(j + 1) * P],
                                             attnT_sb[:, s, sl], start=(s == 0), stop=(s == NS - 1))
                        nc.vector.tensor_tensor(out=xnewT_sb[:, j, sl], in0=ps,
                                                in1=xqT_sb[:, j, sl], op=ALU.add)
                        nc.vector.tensor_copy(out=xnewTr_sb[:, j, sl], in_=xnewT_sb[:, j, sl])

                for tch in range(TQ // 512):
                    sl = slice(tch * 512, (tch + 1) * 512)
                    ps_x = pcs.tile([1, 512], f32, tag="st_x", name=f"m2_{tch}")
                    ps_2 = pcs.tile([1, 512], f32, tag="st_2", name=f"v2_{tch}")
                    for s in range(NS):
                        nc.tensor.matmul(ps_x, ones_r, xqT_sb[:, s, sl], start=(s == 0), stop=False)
                    for s in range(NS):
                        nc.tensor.matmul(ps_x, wocs_sb[:, s:s + 1], attnT_sb[:, s, sl],
                                         start=False, stop=(s == NS - 1))
                    for s in range(NS):
                        x2 = pcc.tile([P, 512], f32r, tag="x2n", name=f"x2n{tch}_{s}")
                        nc.vector.tensor_tensor(out=x2, in0=xnewT_sb[:, s, sl],
                                                in1=xnewT_sb[:, s, sl], op=ALU.mult)
                        nc.tensor.matmul(ps_2, ones_r, x2, start=(s == 0), stop=(s == NS - 1))
                    nc.vector.tensor_scalar_mul(out=mu2_row[:, sl], in0=ps_x, scalar1=1.0 / C)
                    nc.vector.tensor_scalar_mul(out=rstd2_row[:, sl], in0=ps_2, scalar1=1.0 / C)
                nc.vector.tensor_tensor(out=t2_row, in0=mu2_row, in1=mu2_row, op=ALU.mult)
                nc.vector.tensor_tensor(out=rstd2_row, in0=rstd2_row, in1=t2_row, op=ALU.subtract)
                nc.scalar.activation(out=rstd2_row, in_=rstd2_row, func=AF.Ln, bias=eps_sb)
                nc.scalar.activation(out=rstd2_row, in_=rstd2_row, func=AF.Exp, scale=-0.5)
                nc.gpsimd.partition_broadcast(r2b_sb, rstd2_row)

            # ================= Phase D: FFN =================
            with tc.tile_pool(name="pD", bufs=1) as pd, \
                 tc.tile_pool(name="pDy", bufs=3) as pdy, \
                 tc.tile_pool(name="pDps", bufs=4, space="PSUM") as pdp:
                for tch in range(TQ // 512):
                    sl = slice(tch * 512, (tch + 1) * 512)
                    aT = pd.tile([P, F // P, 512], f32r, tag="aT", name=f"aT{tch}")
                    for fj in range(F // P):
                        ps = pdp.tile([P, 512], f32, tag="ff", name=f"ff1_{tch}_{fj}")
                        for s in range(NS):
                            nc.tensor.matmul(ps, w1_sb[:, s, fj * P:(fj + 1) * P],
                                             xnewTr_sb[:, s, sl], start=(s == 0), stop=False)
                        nc.tensor.matmul(ps, nw1cs_sb[:, fj * P:(fj + 1) * P], mu2_row[:, sl],
                                         start=False, stop=True)
                        nc.scalar.activation(out=aT[:, fj], in_=ps, func=AF.Relu)
                    for j in range(NS):
                        ps = pdp.tile([P, 512], f32, tag="ff", name=f"ff2_{tch}_{j}")
                        for fj in range(F // P):
                            nc.tensor.matmul(ps, w2_sb[:, fj, j * P:(j + 1) * P], aT[:, fj],
                                             start=(fj == 0), stop=(fj == F // P - 1))
                        yt = pdy.tile([P, 512], f32, tag="yt", name=f"y{tch}_{j}")
                        nc.vector.tensor_tensor(out=yt, in0=ps, in1=r2b_sb[:, sl], op=ALU.mult)
                        nc.vector.tensor_tensor(out=yt, in0=yt, in1=xnewT_sb[:, j, sl], op=ALU.add)
                        nc.sync.dma_start(out=yT.ap()[j * P:(j + 1) * P, sl], in_=yt)

    with tile.TileContext(nc) as tc, contextlib.ExitStack() as top:
        _body(tc, top)
    nc.finalize()
    return nc


_prog = None


def _get_program():
    global _prog
    if _prog is None:
        _prog = _build_program(os.environ.get("KPH", "full"))
    return _prog


def _host_prep(x, Wq, Wk, Wv, Wo, bo, g1, b1, g2, b2, W_ff1, b_ff1, W_ff2, b_ff2):
    x = np.asarray(x, np.float32)
    for nm, v in (("bo", bo), ("b1", b1), ("b2", b2), ("b_ff1", b_ff1), ("b_ff2", b_ff2)):
        if not np.allclose(np.asarray(v), 0.0):
            raise NotImplementedError(f"nonzero bias {nm} not supported")
    g1 = np.asarray(g1, np.float32)
    g2 = np.asarray(g2, np.float32)
    scale = np.float32(np.float64(C) ** -0.5)
    wqT = np.ascontiguousarray((np.asarray(Wq) * (g1 * scale)[None, :]).T.astype(np.float32))
    wkT = np.ascontiguousarray((np.asarray(Wk) * g1[None, :]).T.astype(np.float32))
    wvT = np.ascontiguousarray((np.asarray(Wv) * g1[None, :]).T.astype(np.float32))
    woT = np.ascontiguousarray(np.asarray(Wo).T.astype(np.float32))
    w1T = np.ascontiguousarray((np.asarray(W_ff1) * g2[None, :]).T.astype(np.float32))
    w2T = np.ascontiguousarray(np.asarray(W_ff2).T.astype(np.float32))
    shared = dict(
        wqT=wqT, wkT=wkT, wvT=wvT, woT=woT, w1T=w1T, w2T=w2T,
        nwqcs=np.ascontiguousarray(-wqT.sum(0)[None, :]),
        nwkcs=np.ascontiguousarray(-wkT.sum(0)[None, :]),
        nwvcs=np.ascontiguousarray(-wvT.sum(0)[None, :]),
        nw1cs=np.ascontiguousarray(-w1T.sum(0)[None, :]),
        wocs=np.ascontiguousarray(np.asarray(Wo).sum(0).astype(np.float32).reshape(NS, P).T),
        cst=np.ones((P, P), np.float32),
    )
    in_maps = []
    for core in range(8):
        b, g = core // 2, core % 2
        chunks = [2 * i + g for i in range(NSLOT)]
        qrows = np.concatenate([np.arange(QC * ch, QC * (ch + 1)) for ch in chunks])
        m = np.zeros((P, NSLOT, 4, QC), np.float32)
        for i, ch in enumerate(chunks):
            for kr in range(4):
                kt = (EXTS[i] // P - 4) + kr
                k_abs = P * kt + np.arange(P)[:, None]
                q_abs = QC * ch + np.arange(QC)[None, :]
                m[:, i, kr, :] = (k_abs <= q_abs).astype(np.float32)
        in_maps.append(dict(
            shared,
            xkT=np.ascontiguousarray(x[b].T),
            xqT=np.ascontiguousarray(x[b][qrows].T),
            masks=m,
        ))
    return in_maps


def kernel(**inputs):
    global _last_exec_time_ns, _last_results
    inputs = {k: np.asarray(v) for k, v in inputs.items()}
    in_maps = _host_prep(**inputs)
    nc = _get_program()
    trace = os.environ.get("KERNEL_TRACE", "0") == "1"
    res = run_bass_kernel_spmd(nc, in_maps, list(range(8)), trace=trace)
    _last_exec_time_ns = res.exec_time_ns
    _last_results = res
    out = np.empty((4, T, C), np.float32)
    for core in range(8):
        b, g = core // 2, core % 2
        yt = res.results[core]["yT"]
        for i in range(NSLOT):
            ch = 2 * i + g
            out[b, QC * ch:QC * (ch + 1), :] = yt[:, QC * i:QC * (i + 1)].T
    return out



# revision 9
# speedup vs baseline: 1.2582x; 1.2582x over previous
"""Trainium2 Bass kernel for a pre-LN transformer block (B=4, T=2048, C=512, H=8).

Sharding: 8 cores, 2 per batch element. Each core handles 4 q-chunks of 256
tokens (core group g takes chunks {2i+g}), with causal k-extents padded to a
uniform schedule {512, 1024, 1536, 2048} so all cores run the same SPMD
program; padding + the causal diagonal are neutralized by multiplicative
{0,1} masks fed as per-core data (last 4 k-tiles of each slot).

On-device math:
  LN stats via ones-matmuls (mean) and squared ones-matmuls (var); rstd via
  exp(-0.5*ln(var+eps)). The LN mean-subtraction is folded into QKV / FF1
  projections as a K=1 rank-1 correction matmul (-colsum(W) (x) mu); the
  per-token rstd is fused into PSUM-evacuation multiplies. g1/g2 are folded
  into the weights host-side; all biases are zero (asserted).
Attention: S^T = K_h^T q with 2 heads packed per 128-row PE pass, exp on ACT
  over 2-bank PSUM groups, masks, then AV via token-major V augmented with a
  ones column so the softmax denominator falls out of the same matmul (M=65).
Attention runs in two sequential head-pair groups to halve K/V/Q residency.
"""

import os
import sys

sys.path.insert(0, "/opt/trn_rl_repo")

import contextlib

import numpy as np

import concourse.bass as bass
import concourse.tile as tile
from concourse import bacc, library_config, mybir
from concourse.bass_utils import run_bass_kernel_spmd

P = 128
C = 512
T = 2048
TQ = 1024
H = 8
HS = 64
F = 2048
NS = 4            # c-subtiles of C
NSLOT = 4         # q-chunks (slots) per core, 256 tokens each
QC = 256          # q-chunk width
EXTS = [512, 1024, 1536, 2048]   # scheduled k-extent per slot
EPS = 1e-5

f32 = mybir.dt.float32
f32r = mybir.dt.float32r
AF = mybir.ActivationFunctionType
ALU = mybir.AluOpType

_last_exec_time_ns = None
_last_results = None


def _build_program(limit="full"):
    nc = bacc.Bacc(name="block")

    def inp(name, shape):
        return nc.declare_dram_parameter(name, list(shape), f32, isOutput=False)

    xkT = inp("xkT", (C, T))          # x[b].T
    xqT = inp("xqT", (C, TQ))         # q-rows of x[b], transposed, slot order
    wqT = inp("wqT", (C, C))          # (Wq*g1).T * C^-0.5
    wkT = inp("wkT", (C, C))
    wvT = inp("wvT", (C, C))
    woT = inp("woT", (C, C))
    w1T = inp("w1T", (C, F))          # (W_ff1*g2).T
    w2T = inp("w2T", (F, C))
    nwqcs = inp("nwqcs", (1, C))      # -colsum(wqT)
    nwkcs = inp("nwkcs", (1, C))
    nwvcs = inp("nwvcs", (1, C))
    nw1cs = inp("nw1cs", (1, F))
    wocs = inp("wocs", (P, NS))       # colsum_j Wo[j, c'] as column tiles
    masks = inp("masks", (P, NSLOT, 4, QC))  # last-4 kt masks per slot
    cst = inp("cst", (P, P))          # ones
    yT = nc.declare_dram_parameter("yT", [C, TQ], f32, isOutput=True)
    scr = nc.dram_tensor("scratch_rk", [1, T], f32)

    def _body(tc, top):
        # pin gpsimd to the proxy library (tensor_tensor + partition_broadcast
        # both resident) — the default per-op assignment reloads ucode (~6.5us
        # stall) at every mask-multiply <-> broadcast switch
        nc.gpsimd.load_library(library_config.proxy)
        # ---------- whole-kernel persistent pool (small) ----------
        pc = top.enter_context(tc.tile_pool(name="const", bufs=1))
        ones_r = pc.tile([P, 1], f32r, tag="ones_r")
        nc.sync.dma_start(out=ones_r, in_=cst.ap()[:, 0:1].bitcast(f32r))
        eps_sb = pc.tile([1, 1], f32, tag="eps")
        nc.vector.memset(eps_sb, EPS)
        wocs_sb = pc.tile([P, NS], f32r, tag="wocs")
        nc.sync.dma_start(out=wocs_sb, in_=wocs.ap().bitcast(f32r))

        # ---------- spans A..C: xqT; spans B..C: attnT ----------
        pAC = top.enter_context(tc.tile_pool(name="pAC", bufs=1))
        xqT_sb = pAC.tile([P, NS, TQ], f32r, tag="xqT")          # 16KB
        for s in range(NS):
            nc.sync.dma_start(out=xqT_sb[:, s], in_=xqT.ap()[s * P:(s + 1) * P, :].bitcast(f32r))
        attnT_sb = pAC.tile([P, NS, TQ], f32r, tag="attnT")      # 16KB

        # ============ stats + per-group (projections + attention) ============
        with tc.tile_pool(name="pStats", bufs=1) as pst_sb, \
             tc.tile_pool(name="pW", bufs=1) as pw, \
             tc.tile_pool(name="pStream", bufs=3) as pstr, \
             tc.tile_pool(name="pX2", bufs=2) as px2:

            # QKV weights (24KB)
            wq_sb = pw.tile([P, NS, C], f32r, tag="wq")
            wk_sb = pw.tile([P, NS, C], f32r, tag="wk")
            wv_sb = pw.tile([P, NS, C], f32r, tag="wv")
            for s in range(NS):
                nc.sync.dma_start(out=wq_sb[:, s], in_=wqT.ap()[s * P:(s + 1) * P, :].bitcast(f32r))
                nc.sync.dma_start(out=wk_sb[:, s], in_=wkT.ap()[s * P:(s + 1) * P, :].bitcast(f32r))
                nc.sync.dma_start(out=wv_sb[:, s], in_=wvT.ap()[s * P:(s + 1) * P, :].bitcast(f32r))
            # masks (16KB), span both groups
            mask_sb = pw.tile([P, NSLOT, 4, QC], f32r, tag="masks")
            for sl_ in range(NSLOT):
                nc.sync.dma_start(out=mask_sb[:, sl_], in_=masks.ap()[:, sl_].bitcast(f32r))

            # stats rows (span both groups)
            muk_row = pst_sb.tile([1, T], f32r, tag="muk")       # 8KB
            rstdk_row = pst_sb.tile([1, T], f32, tag="rstdk")    # 8KB (also e2 dest)
            tmp_row = pst_sb.tile([1, T], f32, tag="tmprow")     # 8KB
            muq_row = pst_sb.tile([1, TQ], f32r, tag="muq")      # 4KB
            rstdq_row = pst_sb.tile([1, TQ], f32, tag="rstdq")   # 4KB
            nwqcs_sb = pst_sb.tile([1, C], f32r, tag="nwqcs")
            nc.sync.dma_start(out=nwqcs_sb, in_=nwqcs.ap().bitcast(f32r))
            nwkcs_sb = pst_sb.tile([1, C], f32r, tag="nwkcs")
            nc.sync.dma_start(out=nwkcs_sb, in_=nwkcs.ap().bitcast(f32r))
            nwvcs_sb = pst_sb.tile([1, C], f32r, tag="nwvcs")
            nc.sync.dma_start(out=nwvcs_sb, in_=nwvcs.ap().bitcast(f32r))
            rkb_sb = pst_sb.tile([P, T], f32, tag="rkb")         # 8KB
            rqb_sb = pst_sb.tile([P, TQ], f32, tag="rqb")        # 4KB
            rstdk_col = pst_sb.tile([P, T // P], f32, tag="rkcol")

            # ---- LN1 stats (xk streamed, xq resident) ----
            with tc.tile_pool(name="pStPs", bufs=4, space="PSUM") as pstp:
                for tch in range(T // 512):
                    sl = slice(tch * 512, (tch + 1) * 512)
                    ps_x = pstp.tile([1, 512], f32, tag="st_x", name=f"stxk{tch}")
                    ps_2 = pstp.tile([1, 512], f32, tag="st_2", name=f"st2k{tch}")
                    for s in range(NS):
                        xs = pstr.tile([P, 512], f32r, tag="xks", name=f"xks{tch}_{s}")
                        nc.sync.dma_start(out=xs, in_=xkT.ap()[s * P:(s + 1) * P, sl].bitcast(f32r))
                        nc.tensor.matmul(ps_x, ones_r, xs, start=(s == 0), stop=(s == NS - 1))
                        x2 = px2.tile([P, 512], f32r, tag="x2", name=f"x2k{tch}_{s}")
                        nc.vector.tensor_tensor(out=x2, in0=xs, in1=xs, op=ALU.mult)
                        nc.tensor.matmul(ps_2, ones_r, x2, start=(s == 0), stop=(s == NS - 1))
                    nc.vector.tensor_scalar_mul(out=muk_row[:, sl], in0=ps_x, scalar1=1.0 / C)
                    nc.vector.tensor_scalar_mul(out=rstdk_row[:, sl], in0=ps_2, scalar1=1.0 / C)
                for tch in range(TQ // 512):
                    sl = slice(tch * 512, (tch + 1) * 512)
                    ps_x = pstp.tile([1, 512], f32, tag="st_x", name=f"stxq{tch}")
                    ps_2 = pstp.tile([1, 512], f32, tag="st_2", name=f"st2q{tch}")
                    for s in range(NS):
                        nc.tensor.matmul(ps_x, ones_r, xqT_sb[:, s, sl],
                                         start=(s == 0), stop=(s == NS - 1))
                        x2 = px2.tile([P, 512], f32r, tag="x2", name=f"x2q{tch}_{s}")
                        nc.vector.tensor_tensor(out=x2, in0=xqT_sb[:, s, sl],
                                                in1=xqT_sb[:, s, sl], op=ALU.mult)
                        nc.tensor.matmul(ps_2, ones_r, x2, start=(s == 0), stop=(s == NS - 1))
                    nc.vector.tensor_scalar_mul(out=muq_row[:, sl], in0=ps_x, scalar1=1.0 / C)
                    nc.vector.tensor_scalar_mul(out=rstdq_row[:, sl], in0=ps_2, scalar1=1.0 / C)

            def finish_rstd(mu_row, rstd_row, t_row):
                # rstd <- exp(-0.5*ln((E[x^2] - mu^2) + eps)); rstd_row holds E[x^2]
                nc.vector.tensor_tensor(out=t_row, in0=mu_row, in1=mu_row, op=ALU.mult)
                nc.vector.tensor_tensor(out=rstd_row, in0=rstd_row, in1=t_row, op=ALU.subtract)
                nc.scalar.activation(out=rstd_row, in_=rstd_row, func=AF.Ln, bias=eps_sb)
                nc.scalar.activation(out=rstd_row, in_=rstd_row, func=AF.Exp, scale=-0.5)

            finish_rstd(muk_row, rstdk_row, tmp_row)
            finish_rstd(muq_row, rstdq_row, tmp_row[:, 0:TQ])
            nc.gpsimd.partition_broadcast(rkb_sb, rstdk_row)
            nc.gpsimd.partition_broadcast(rqb_sb, rstdq_row)
            # partition-scatter sbuf->sbuf DMA corrupts data on HW; round-trip
            # through DRAM, whose DMA distributes across partitions correctly
            nc.sync.dma_start(out=scr.ap(), in_=rstdk_row)
            nc.sync.dma_start(out=rstdk_col, in_=scr.ap().rearrange("a (o p) -> (a p) o", p=P))
            if limit == "stats":
                nc.sync.dma_start(out=yT.ap()[0:1, 0:TQ], in_=rstdk_row[:, 0:TQ])
                nc.sync.dma_start(out=yT.ap()[1:2, 0:TQ], in_=rstdk_row[:, TQ:T])
                nc.sync.dma_start(out=yT.ap()[2:3, 0:TQ], in_=muk_row[:, 0:TQ].bitcast(f32))
                nc.sync.dma_start(out=yT.ap()[3:4, 0:TQ], in_=muk_row[:, TQ:T].bitcast(f32))
                nc.sync.dma_start(out=yT.ap()[4:4 + P, 0:T // P], in_=rstdk_col)
                return

            # ---- two head-pair groups: projections then attention ----
            for grp in range(2 if limit not in ("proj1", "attn1") else 1):
                with tc.tile_pool(name=f"pG{grp}", bufs=1) as pg:
                    kT_sb = pg.tile([P, 2, T], f32r, tag="kT")           # 16KB
                    vaug_sb = pg.tile([P, T // P, 4 * 65], f32r, tag="vaug")  # 16.6KB
                    qT_sb = pg.tile([P, 2, TQ], f32r, tag="qT")          # 8KB
                    nc.sync.dma_start(
                        out=vaug_sb.rearrange("p t (h x) -> p t h x", x=65)[:, :, :, 64:65],
                        in_=cst.ap()[:, 0:64].rearrange("p (t h x) -> p t h x", h=4, x=1).bitcast(f32r),
                    )

                    with tc.tile_pool(name=f"pPrj{grp}", bufs=3, space="PSUM") as pap:
                        # K projection for this group's 2 feature tiles
                        for jj in range(2):
                            j = 2 * grp + jj
                            for tch in range(T // 512):
                                sl = slice(tch * 512, (tch + 1) * 512)
                                ps = pap.tile([P, 512], f32, tag="proj", name=f"k{grp}_{jj}_{tch}")
                                for s in range(NS):
                                    xs = pstr.tile([P, 512], f32r, tag="xks", name=f"kx{grp}_{jj}_{tch}_{s}")
                                    nc.sync.dma_start(out=xs, in_=xkT.ap()[s * P:(s + 1) * P, sl].bitcast(f32r))
                                    nc.tensor.matmul(ps, wk_sb[:, s, j * P:(j + 1) * P], xs,
                                                     start=(s == 0), stop=False)
                                nc.tensor.matmul(ps, nwkcs_sb[:, j * P:(j + 1) * P], muk_row[:, sl],
                                                 start=False, stop=True)
                                nc.vector.tensor_tensor(out=kT_sb[:, jj, sl], in0=ps,
                                                        in1=rkb_sb[:, sl], op=ALU.mult)
                        # V projection (token-major into V_aug)
                        for tt in range(T // P):
                            tsl = slice(tt * P, (tt + 1) * P)
                            ps = pap.tile([P, 256], f32, tag="projv", name=f"v{grp}_{tt}")
                            xs = pstr.tile([P, NS, P], f32r, tag="xkv", name=f"vx{grp}_{tt}")
                            for s2 in range(NS):
                                nc.sync.dma_start(
                                    out=xs[:, s2],
                                    in_=xkT.ap()[s2 * P:(s2 + 1) * P, tsl].bitcast(f32r))
                            for s in range(NS):
                                nc.tensor.matmul(ps, xs[:, s], wv_sb[:, s, 256 * grp:256 * (grp + 1)],
                                                 start=(s == 0), stop=False)
                            nc.tensor.matmul(ps, muk_row[:, tsl], nwvcs_sb[:, 256 * grp:256 * (grp + 1)],
                                             start=False, stop=True)
                            nc.vector.tensor_scalar_mul(
                                out=vaug_sb[:, tt].rearrange("p (h x) -> p h x", x=65)[:, :, 0:64],
                                in0=ps.rearrange("p (h d) -> p h d", d=HS),
                                scalar1=rstdk_col[:, tt:tt + 1])
                        # Q projection
                        for jj in range(2):
                            j = 2 * grp + jj
                            for tch in range(TQ // 512):
                                sl = slice(tch * 512, (tch + 1) * 512)
                                ps = pap.tile([P, 512], f32, tag="proj", name=f"q{grp}_{jj}_{tch}")
                                for s in range(NS):
                                    nc.tensor.matmul(ps, wq_sb[:, s, j * P:(j + 1) * P],
                                                     xqT_sb[:, s, sl], start=(s == 0), stop=False)
                                nc.tensor.matmul(ps, nwqcs_sb[:, j * P:(j + 1) * P], muq_row[:, sl],
                                                 start=False, stop=True)
                                nc.vector.tensor_tensor(out=qT_sb[:, jj, sl], in0=ps,
                                                        in1=rqb_sb[:, sl], op=ALU.mult)

                    if limit in ("proj1", "proj"):
                        nc.sync.dma_start(out=yT.ap()[0:P, 0:TQ], in_=kT_sb[:, 0, 0:TQ].bitcast(f32))
                        continue
                    # ---- attention for this group's 2 head-pairs ----
                    with tc.tile_pool(name=f"pP{grp}", bufs=3) as pp, \
                         tc.tile_pool(name=f"pEps{grp}", bufs=3) as pe, \
                         tc.tile_pool(name=f"pSps{grp}", bufs=2, space="PSUM") as pbp, \
                         tc.tile_pool(name=f"pAVps{grp}", bufs=2, space="PSUM") as pbo:
                        for jj in range(2):
                            hp = 2 * grp + jj
                            for slot in range(NSLOT):
                                nkt = EXTS[slot] // P
                                qsl = slice(slot * QC, (slot + 1) * QC)
                                # one accumulator bank per head: interleaved
                                # accumulation groups must not share a bank
                                # (start=True clears the whole bank's
                                # has_written bits)
                                po = [pbo.tile([65, QC], f32, tag=f"av{hi}",
                                               name=f"av{hp}_{slot}_{hi}")
                                      for hi in range(2)]
                                pending = None

                                def emit_av(ktp, p_tile, po=po, nkt=nkt):
                                    for i in range(2):
                                        kt = 2 * ktp + i
                                        for hi in range(2):
                                            h_loc = 2 * jj + hi
                                            nc.tensor.matmul(
                                                po[hi],
                                                vaug_sb[:, kt, h_loc * 65:(h_loc + 1) * 65],
                                                p_tile[:, hi, i, :],
                                                start=(kt == 0),
                                                stop=(kt == nkt - 1),
                                            )

                                for ktp in range(nkt // 2):
                                    # psum layout [P, head, kt-parity, QC]: each
                                    # bank hosts a single PE row-group — base-0
                                    # and base-64 matmuls sharing a bank return
                                    # garbage on HW at scale
                                    sp = pbp.tile([P, 2, 2, QC], f32, tag="spair",
                                                  name=f"s{hp}_{slot}_{ktp}")
                                    for i in range(2):
                                        kt = 2 * ktp + i
                                        ksl = slice(kt * P, (kt + 1) * P)
                                        nc.tensor.matmul(sp[:, 0, i, :], kT_sb[0:64, jj, ksl],
                                                         qT_sb[0:64, jj, qsl], start=True, stop=True)
                                        nc.tensor.matmul(sp[:, 1, i, :], kT_sb[64:128, jj, ksl],
                                                         qT_sb[64:128, jj, qsl], start=True, stop=True)
                                    pt = pp.tile([P, 2, 2, QC], f32r, tag="p",
                                                 name=f"p{hp}_{slot}_{ktp}")
                                    nc.scalar.activation(out=pt, in_=sp, func=AF.Exp)
                                    for i in range(2):
                                        kt = 2 * ktp + i
                                        if kt >= nkt - 4:
                                            eng = nc.vector if (kt % 2 == 0) else nc.gpsimd
                                            m = mask_sb[:, slot, kt - (nkt - 4)]
                                            for hi in range(2):
                                                eng.tensor_tensor(
                                                    out=pt[:, hi, i, :],
                                                    in0=pt[:, hi, i, :],
                                                    in1=m, op=ALU.mult)
                                    if pending is not None:
                                        emit_av(*pending)
                                    pending = (ktp, pt)
                                emit_av(*pending)

                                for hi in range(2):
                                    r_row = pe.tile([1, QC], f32, tag="r",
                                                    name=f"r{hp}_{slot}_{hi}")
                                    # 1/d = exp(-ln d) on the scalar engine —
                                    # keeps the slot-finalize chain off Vector
                                    nc.scalar.activation(out=r_row, in_=po[hi][64:65, :],
                                                         func=AF.Ln)
                                    nc.scalar.activation(out=r_row, in_=r_row,
                                                         func=AF.Exp, scale=-1.0)
                                    rrep = pe.tile([64, QC], f32, tag="rrep",
                                                   name=f"rr{hp}_{slot}_{hi}")
                                    nc.gpsimd.partition_broadcast(rrep, r_row)
                                    nc.vector.tensor_tensor(
                                        out=attnT_sb[hi * 64:(hi + 1) * 64, hp, qsl],
                                        in0=po[hi][0:64, :], in1=rrep, op=ALU.mult)

        if limit in ("proj1", "proj", "attn1", "attn"):
            if limit in ("attn1", "attn"):
                for s in range(NS):
                    nc.sync.dma_start(out=yT.ap()[s * P:(s + 1) * P, :], in_=attnT_sb[:, s].bitcast(f32))
            return

        # ================= Phase C: Wo + residual + LN2 stats =================
        with tc.tile_pool(name="pCD", bufs=1) as pcd, \
             tc.tile_pool(name="pCDrows", bufs=1) as pcr:
            wo_sb = pcd.tile([P, NS, C], f32r, tag="wo")
            for s in range(NS):
                nc.sync.dma_start(out=wo_sb[:, s], in_=woT.ap()[s * P:(s + 1) * P, :].bitcast(f32r))
            w1_sb = pcd.tile([P, NS, F], f32r, tag="w1")
            for s in range(NS):
                nc.sync.dma_start(out=w1_sb[:, s], in_=w1T.ap()[s * P:(s + 1) * P, :].bitcast(f32r))
            w2_sb = pcd.tile([P, F // P, C], f32r, tag="w2")
            for s in range(F // P):
                nc.sync.dma_start(out=w2_sb[:, s], in_=w2T.ap()[s * P:(s + 1) * P, :].bitcast(f32r))
            xnewT_sb = pcd.tile([P, NS, TQ], f32, tag="xnewT")
            xnewTr_sb = pcd.tile([P, NS, TQ], f32r, tag="xnewTr")
            mu2_row = pcr.tile([1, TQ], f32r, tag="mu2")
            rstd2_row = pcr.tile([1, TQ], f32, tag="rstd2")
            t2_row = pcr.tile([1, TQ], f32, tag="t2row")
            nw1cs_sb = pcr.tile([1, F], f32r, tag="nw1cs")
            nc.sync.dma_start(out=nw1cs_sb, in_=nw1cs.ap().bitcast(f32r))
            r2b_sb = pcr.tile([P, TQ], f32, tag="r2b")

            with tc.tile_pool(name="pC2", bufs=2) as pcc, \
                 tc.tile_pool(name="pCps", bufs=3, space="PSUM") as pcp, \
                 tc.tile_pool(name="pCst", bufs=2, space="PSUM") as pcs:
                for j in range(NS):
                    for tch in range(TQ // 512):
                        sl = slice(tch * 512, (tch + 1) * 512)
                        ps = pcp.tile([P, 512], f32, tag="proj", name=f"wo{j}_{tch}")
                        for s in range(NS):
                            nc.tensor.matmul(ps, wo_sb[:, s, j * P:(j + 1) * P],
                                             attnT_sb[:, s, sl], start=(s == 0), stop=(s == NS - 1))
                        nc.vector.tensor_tensor(out=xnewT_sb[:, j, sl], in0=ps,
                                                in1=xqT_sb[:, j, sl], op=ALU.add)
                        nc.vector.tensor_copy(out=xnewTr_sb[:, j, sl], in_=xnewT_sb[:, j, sl])

                for tch in range(TQ // 512):
                    sl = slice(tch * 512, (tch + 1) * 512)
                    ps_x = pcs.tile([1, 512], f32, tag="st_x", name=f"m2_{tch}")
                    ps_2 = pcs.tile([1, 512], f32, tag="st_2", name=f"v2_{tch}")
                    for s in range(NS):
                        nc.tensor.matmul(ps_x, ones_r, xqT_sb[:, s, sl], start=(s == 0), stop=False)
                    for s in range(NS):
                        nc.tensor.matmul(ps_x, wocs_sb[:, s:s + 1], attnT_sb[:, s, sl],
                                         start=False, stop=(s == NS - 1))
                    for s in range(NS):
                        x2 = pcc.tile([P, 512], f32r, tag="x2n", name=f"x2n{tch}_{s}")
                        nc.vector.tensor_tensor(out=x2, in0=xnewT_sb[:, s, sl],
                                                in1=xnewT_sb[:, s, sl], op=ALU.mult)
                        nc.tensor.matmul(ps_2, ones_r, x2, start=(s == 0), stop=(s == NS - 1))
                    nc.vector.tensor_scalar_mul(out=mu2_row[:, sl], in0=ps_x, scalar1=1.0 / C)
                    nc.vector.tensor_scalar_mul(out=rstd2_row[:, sl], in0=ps_2, scalar1=1.0 / C)
                nc.vector.tensor_tensor(out=t2_row, in0=mu2_row, in1=mu2_row, op=ALU.mult)
                nc.vector.tensor_tensor(out=rstd2_row, in0=rstd2_row, in1=t2_row, op=ALU.subtract)
                nc.scalar.activation(out=rstd2_row, in_=rstd2_row, func=AF.Ln, bias=eps_sb)
                nc.scalar.activation(out=rstd2_row, in_=rstd2_row, func=AF.Exp, scale=-0.5)
                nc.gpsimd.partition_broadcast(r2b_sb, rstd2_row)

            # ================= Phase D: FFN =================
            with tc.tile_pool(name="pD", bufs=1) as pd, \
                 tc.tile_pool(name="pDy", bufs=3) as pdy, \
                 tc.tile_pool(name="pDps", bufs=4, space="PSUM") as pdp:
                for tch in range(TQ // 512):
                    sl = slice(tch * 512, (tch + 1) * 512)
                    aT = pd.tile([P, F // P, 512], f32r, tag="aT", name=f"aT{tch}")
                    for fj in range(F // P):
                        ps = pdp.tile([P, 512], f32, tag="ff", name=f"ff1_{tch}_{fj}")
                        for s in range(NS):
                            nc.tensor.matmul(ps, w1_sb[:, s, fj * P:(fj + 1) * P],
                                             xnewTr_sb[:, s, sl], start=(s == 0), stop=False)
                        nc.tensor.matmul(ps, nw1cs_sb[:, fj * P:(fj + 1) * P], mu2_row[:, sl],
                                         start=False, stop=True)
                        nc.scalar.activation(out=aT[:, fj], in_=ps, func=AF.Relu)
                    for j in range(NS):
                        ps = pdp.tile([P, 512], f32, tag="ff", name=f"ff2_{tch}_{j}")
                        for fj in range(F // P):
                            nc.tensor.matmul(ps, w2_sb[:, fj, j * P:(j + 1) * P], aT[:, fj],
                                             start=(fj == 0), stop=(fj == F // P - 1))
                        yt = pdy.tile([P, 512], f32, tag="yt", name=f"y{tch}_{j}")
                        nc.vector.tensor_tensor(out=yt, in0=ps, in1=r2b_sb[:, sl], op=ALU.mult)
                        nc.vector.tensor_tensor(out=yt, in0=yt, in1=xnewT_sb[:, j, sl], op=ALU.add)
                        nc.sync.dma_start(out=yT.ap()[j * P:(j + 1) * P, sl], in_=yt)

    with tile.TileContext(nc) as tc, contextlib.ExitStack() as top:
        _body(tc, top)
    nc.finalize()
    return nc


_prog = None


def _get_program():
    global _prog
    if _prog is None:
        _prog = _build_program(os.environ.get("KPH", "full"))
    return _prog


def _host_prep(x, Wq, Wk, Wv, Wo, bo, g1, b1, g2, b2, W_ff1, b_ff1, W_ff2, b_ff2):
    x = np.asarray(x, np.float32)
    for nm, v in (("bo", bo), ("b1", b1), ("b2", b2), ("b_ff1", b_ff1), ("b_ff2", b_ff2)):
        if not np.allclose(np.asarray(v), 0.0):
            raise NotImplementedError(f"nonzero bias {nm} not supported")
    g1 = np.asarray(g1, np.float32)
    g2 = np.asarray(g2, np.float32)
    scale = np.float32(np.float64(C) ** -0.5)
    wqT = np.ascontiguousarray((np.asarray(Wq) * (g1 * scale)[None, :]).T.astype(np.float32))
    wkT = np.ascontiguousarray((np.asarray(Wk) * g1[None, :]).T.astype(np.float32))
    wvT = np.ascontiguousarray((np.asarray(Wv) * g1[None, :]).T.astype(np.float32))
    woT = np.ascontiguousarray(np.asarray(Wo).T.astype(np.float32))
    w1T = np.ascontiguousarray((np.asarray(W_ff1) * g2[None, :]).T.astype(np.float32))
    w2T = np.ascontiguousarray(np.asarray(W_ff2).T.astype(np.float32))
    shared = dict(
        wqT=wqT, wkT=wkT, wvT=wvT, woT=woT, w1T=w1T, w2T=w2T,
        nwqcs=np.ascontiguousarray(-wqT.sum(0)[None, :]),
        nwkcs=np.ascontiguousarray(-wkT.sum(0)[None, :]),
        nwvcs=np.ascontiguousarray(-wvT.sum(0)[None, :]),
        nw1cs=np.ascontiguousarray(-w1T.sum(0)[None, :]),
        wocs=np.ascontiguousarray(np.asarray(Wo).sum(0).astype(np.float32).reshape(NS, P).T),
        cst=np.ones((P, P), np.float32),
    )
    in_maps = []
    for core in range(8):
        b, g = core // 2, core % 2
        chunks = [2 * i + g for i in range(NSLOT)]
        qrows = np.concatenate([np.arange(QC * ch, QC * (ch + 1)) for ch in chunks])
        m = np.zeros((P, NSLOT, 4, QC), np.float32)
        for i, ch in enumerate(chunks):
            for kr in range(4):
                kt = (EXTS[i] // P - 4) + kr
                k_abs = P * kt + np.arange(P)[:, None]
                q_abs = QC * ch + np.arange(QC)[None, :]
                m[:, i, kr, :] = (k_abs <= q_abs).astype(np.float32)
        in_maps.append(dict(
            shared,
            xkT=np.ascontiguousarray(x[b].T),
            xqT=np.ascontiguousarray(x[b][qrows].T),
            masks=m,
        ))
    return in_maps


def kernel(**inputs):
    global _last_exec_time_ns, _last_results
    inputs = {k: np.asarray(v) for k, v in inputs.items()}
    in_maps = _host_prep(**inputs)
    nc = _get_program()
    trace = os.environ.get("KERNEL_TRACE", "0") == "1"
    res = run_bass_kernel_spmd(nc, in_maps, list(range(8)), trace=trace)
    _last_exec_time_ns = res.exec_time_ns
    _last_results = res
    out = np.empty((4, T, C), np.float32)
    for core in range(8):
        b, g = core // 2, core % 2
        yt = res.results[core]["yT"]
        for i in range(NSLOT):
            ch = 2 * i + g
            out[b, QC * ch:QC * (ch + 1), :] = yt[:, QC * i:QC * (i + 1)].T
    return out



# revision 34
# speedup vs baseline: 2.5209x; 2.0036x over previous
"""Trainium2 Bass kernel for a pre-LN transformer block (B=4, T=2048, C=512, H=8).

Sharding: 8 cores, 2 per batch element. Core group g takes q-chunks {2i+g}
(256 tokens each) with causal k-extents padded to the uniform schedule
{512, 1024, 1536, 2048}; the causal diagonal and schedule padding are
neutralized by additive -3e4 masks applied to S in PSUM before exp.

v2: fp8e4m3 DoubleRow matmuls (0.5 cyc/row) for QKV/Wo/FFN projections and
AV; bf16 for the S=K^T q matmuls. x is centered on-device (x - mu, fp8)
which removes all rank-1 mean-correction matmuls; rstd (and all fp8 dequant
scales) are folded into the PSUM-evacuation multiplies / Exp biases.
Weights are pre-scaled by 64 host-side so their fp8 encodings stay in the
normal range. Softmax: pt = exp(S + ln 64) in fp8 feeds DoubleRow AV with a
token-major V augmented with a ones column, so numerator and denominator
come out of one matmul; denominators are normalized via a per-head-pair
batched reciprocal (DRAM roundtrip to spread rows across partitions).
"""

import os
import sys

sys.path.insert(0, "/opt/trn_rl_repo")

import contextlib

import numpy as np
import ml_dtypes

import importlib.util

if importlib.util.find_spec("antenv") is not None \
        and importlib.util.find_spec("antenv.axon_hooks") is None:
    # bass_utils imports antenv.axon_hooks when tracing is requested; stub it
    # so a BASS_TRACE env var can't crash the run in minimal containers
    import types

    _ah = types.ModuleType("antenv.axon_hooks")
    _ah._hook = None
    _ah.set_axon_ntff_profile_hook = lambda h: setattr(_ah, "_hook", h)
    _ah.get_axon_ntff_profile_hook = lambda: _ah._hook
    sys.modules["antenv.axon_hooks"] = _ah

import concourse.bass as bass
import concourse.tile as tile
from concourse import bacc, library_config, mybir
from concourse.bass_utils import run_bass_kernel_spmd

P = 128
C = 512
T = 2048
TQ = 1024
H = 8
HS = 64
F = 2048
NS = 4            # c-subtiles of C
NSLOT = 4         # q-chunks (slots) per core, 256 tokens each
QC = 256          # q-chunk width
EXTS = [512, 1024, 1536, 2048]   # scheduled k-extent per slot
EPS = 1e-5
WS = 64.0         # fp8 weight pre-scale
LNWS = float(np.log(WS))

f32 = mybir.dt.float32
f32r = mybir.dt.float32r
bf16 = mybir.dt.bfloat16
fp8 = mybir.dt.float8e4
AF = mybir.ActivationFunctionType
ALU = mybir.AluOpType
DR = mybir.MatmulPerfMode.DoubleRow

_last_exec_time_ns = None
_last_results = None


def _build_program():
    from concourse.hw_specs import get_activation_tables
    nc = bacc.Bacc(name="block2")
    # The act-table pass picks the first set containing each function, which
    # thrashes ACT_TABLE_LOADs between the exp and ln tables. Narrow the
    # (cached) table map so natural_log_exp_and_others is the only provider
    # of the functions this kernel uses -> a single table load at the top.
    tabs = get_activation_tables(nc.m.arch)
    _keep = {AF.Exp, AF.Ln, AF.Copy, AF.Square, AF.Relu}
    if "natural_log_exp_and_others" in tabs:
        for _n, _funcs in tabs.items():
            if _n != "natural_log_exp_and_others":
                _funcs -= _keep

    def inp(name, shape, dt=f32):
        return nc.declare_dram_parameter(name, list(shape), dt, isOutput=False)

    xkT = inp("xkT", (C, T))             # x[b].T fp32
    xq = inp("xq", (C, TQ))              # q-rows of x[b], transposed, slot order
    wq8 = inp("wq8", (P, NS, C), fp8)    # (Wq*g1).T * C^-0.5 * 64, [in-ktile, out]
    wk8 = inp("wk8", (P, NS, C), fp8)
    wv8 = inp("wv8", (P, NS, C), fp8)
    wo8 = inp("wo8", (P, NS, C), fp8)
    w18 = inp("w18", (P, NS, F), fp8)
    w28 = inp("w28", (P, F // P, C), fp8)
    masks = inp("masks", (P, NSLOT, 4, 2 * QC), bf16)  # additive, doubled for 2 heads
    ident = inp("ident", (P, P), bf16)   # identity, for mask-add matmuls
    cst = inp("cst", (P, P))             # ones
    yT = nc.declare_dram_parameter("yT", [C, TQ], f32, isOutput=True)
    scr_rk = nc.dram_tensor("scr_rk", [1, T], f32)
    scr_den = nc.dram_tensor("scr_den", [NS, 2 * NSLOT * QC], f32)
    scr_rcp = nc.dram_tensor("scr_rcp", [NS, 2 * NSLOT * QC], f32)

    def _body(tc, top):
        nc.gpsimd.load_library(library_config.proxy)

        # ---------- persistent tensors ----------
        pc = top.enter_context(tc.tile_pool(name="const", bufs=1))
        ones_r = pc.tile([P, 1], f32r, tag="ones_r")
        nc.sync.dma_start(out=ones_r, in_=cst.ap()[:, 0:1].bitcast(f32r))
        eps_sb = pc.tile([1, 1], f32, tag="eps")
        nc.vector.memset(eps_sb, EPS)
        nls_sb = pc.tile([1, 1], f32, tag="nls")
        nc.vector.memset(nls_sb, -LNWS)
        nls2_sb = pc.tile([1, 1], f32, tag="nls2")
        nc.vector.memset(nls2_sb, -2.0 * LNWS)
        lnw_col = pc.tile([P, 1], f32, tag="lnw")
        nc.vector.memset(lnw_col, LNWS)


        pers = top.enter_context(tc.tile_pool(name="pers", bufs=1))
        xc8_sb = pers.tile([P, NS, T], fp8, tag="xc8")       # centered x, 8KB
        kT_sb = pers.tile([P, NS, T], bf16, tag="kT")        # 16KB
        qT_sb = pers.tile([P, NS, TQ], bf16, tag="qT")       # 8KB
        vaug_sb = pers.tile([P, T // P, H, 66], fp8, tag="vaug")  # 8.4KB (66: DR needs even M)
        attnT_sb = pers.tile([P, NS, TQ], fp8, tag="attnT")  # 4KB
        rkcol_sb = pers.tile([P, T // P], f32, tag="rkcol")
        xq_sb = pers.tile([P, NS, TQ], f32r, tag="xq")       # residual + q stats, 16KB
        xcq8_sb = pers.tile([P, NS, TQ], fp8, tag="xcq8")    # centered q-side, 4KB
        mask_sb = pers.tile([P, NSLOT, 4, 2 * QC], bf16, tag="masks")  # 8KB
        id_sb = pers.tile([P, P], bf16, tag="ident")
        nc.vector.memset(vaug_sb.rearrange("p t h x -> p (t h) x")[:, :, 64:65], 1.0)
        nc.vector.memset(vaug_sb.rearrange("p t h x -> p (t h) x")[:, :, 65:66], 0.0)

        pw = top.enter_context(tc.tile_pool(name="pW", bufs=1))
        wq_sb = pw.tile([P, NS, C], fp8, tag="wq")
        wk_sb = pw.tile([P, NS, C], fp8, tag="wk")
        wv_sb = pw.tile([P, NS, C], fp8, tag="wv")
        wo_sb = pw.tile([P, NS, C], fp8, tag="wo")
        w1_sb = pw.tile([P, NS, F], fp8, tag="w1")
        w2_sb = pw.tile([P, F // P, C], fp8, tag="w2")


        # ============ Phase A: stats + center + K/V/Q projections ============
        with tc.tile_pool(name="pStr", bufs=12) as pstr, \
             tc.tile_pool(name="pX2", bufs=4) as px2, \
             tc.tile_pool(name="pRows", bufs=3) as prow, \
             tc.tile_pool(name="pBcast", bufs=3) as pbc, \
             tc.tile_pool(name="pStPs", bufs=2, space="PSUM") as pstp, \
             tc.tile_pool(name="pPrj", bufs=4, space="PSUM") as pap:
            def stats_rows(src_tiles, tag):
                ps_x = pstp.tile([1, 512], f32, tag="st_x", name=f"stx{tag}")
                ps_2 = pstp.tile([1, 512], f32, tag="st_2", name=f"st2{tag}")
                for s in range(NS):
                    nc.tensor.matmul(ps_x, ones_r, src_tiles[s],
                                     start=(s == 0), stop=(s == NS - 1))
                    x2 = px2.tile([P, 512], f32r, tag="x2", name=f"x2{tag}_{s}")
                    nc.scalar.activation(out=x2, in_=src_tiles[s], func=AF.Square)
                    nc.tensor.matmul(ps_2, ones_r, x2, start=(s == 0), stop=(s == NS - 1))
                mu_row = prow.tile([1, 512], f32, tag="mu", name=f"mu{tag}")
                rstd_row = prow.tile([1, 512], f32, tag="rstd", name=f"rstd{tag}")
                t_row = prow.tile([1, 512], f32, tag="trow", name=f"t{tag}")
                nc.vector.tensor_scalar_mul(out=mu_row, in0=ps_x, scalar1=1.0 / C)
                nc.vector.tensor_scalar_mul(out=rstd_row, in0=ps_2, scalar1=1.0 / C)
                # rstd/WS <- exp(-0.5*ln(var+eps) - ln WS)
                nc.vector.tensor_tensor(out=t_row, in0=mu_row, in1=mu_row, op=ALU.mult)
                nc.vector.tensor_tensor(out=rstd_row, in0=rstd_row, in1=t_row, op=ALU.subtract)
                nc.scalar.activation(out=rstd_row, in_=rstd_row, func=AF.Ln, bias=eps_sb)
                nc.scalar.activation(out=rstd_row, in_=rstd_row, func=AF.Exp,
                                     scale=-0.5, bias=nls_sb)
                mub = pbc.tile([P, 512], f32, tag="mub", name=f"mub{tag}")
                rb = pbc.tile([P, 512], f32, tag="rkb", name=f"rb{tag}")
                nc.gpsimd.partition_broadcast(mub, mu_row)
                nc.gpsimd.partition_broadcast(rb, rstd_row)
                return mu_row, rstd_row, mub, rb

            def k_chunk(tch, pre=None):
                sl = slice(tch * 512, (tch + 1) * 512)
                xs = []
                for s in range(NS):
                    xt = pstr.tile([P, 512], f32r, tag="xs", name=f"xs{tch}_{s}")
                    nc.sync.dma_start(out=xt, in_=xkT.ap()[s * P:(s + 1) * P, sl].bitcast(f32r))
                    xs.append(xt)
                if pre is not None:
                    pre()
                mu_row, rstd_row, mub, rkb = stats_rows(xs, f"k{tch}")
                # token-major rstd/WS column tile for the V evacuation
                nc.sync.dma_start(out=scr_rk.ap()[:, sl], in_=rstd_row)
                nc.sync.dma_start(
                    out=rkcol_sb[:, 4 * tch:4 * tch + 4],
                    in_=scr_rk.ap()[:, sl].rearrange("a (o p) -> (a p) o", p=P))
                for s in range(NS):
                    eng = nc.vector if s < 2 else nc.gpsimd
                    eng.tensor_tensor(out=xc8_sb[:, s, sl], in0=xs[s], in1=mub,
                                      op=ALU.subtract)
                # K projection (feature-major)
                for j in range(NS):
                    ps = pap.tile([P, 512], f32, tag="pj", name=f"k{tch}_{j}")
                    for pr in range(2):
                        nc.tensor.matmul(ps, wk_sb[:, 2 * pr:2 * pr + 2, j * P:(j + 1) * P],
                                         xc8_sb[:, 2 * pr:2 * pr + 2, sl],
                                         start=(pr == 0), stop=(pr == 1), perf_mode=DR)
                    nc.vector.tensor_tensor(out=kT_sb[:, j, sl], in0=ps, in1=rkb, op=ALU.mult)
                # V projection (token-major, into V_aug with rstd/WS column scale)
                for tt in range(4):
                    ta = 4 * tch + tt
                    tsl = slice(ta * P, (ta + 1) * P)
                    ps = pap.tile([P, 512], f32, tag="pj", name=f"v{tch}_{tt}")
                    for pr in range(2):
                        nc.tensor.matmul(ps, xc8_sb[:, 2 * pr:2 * pr + 2, tsl],
                                         wv_sb[:, 2 * pr:2 * pr + 2, :],
                                         start=(pr == 0), stop=(pr == 1), perf_mode=DR)
                    nc.scalar.activation(
                        out=vaug_sb[:, ta, :, 0:64],
                        in_=ps.rearrange("p (h d) -> p h d", d=HS),
                        func=AF.Copy, scale=rkcol_sb[:, ta:ta + 1])

            def q_chunk(tch):
                # q-side stats + center + Q projection (uniform across cores:
                # the per-core q-columns arrive as data in xq)
                sl = slice(tch * 512, (tch + 1) * 512)
                xqs = [xq_sb[:, s, sl] for s in range(NS)]
                mu_row, rstd_row, mub, rqb = stats_rows(xqs, f"q{tch}")
                for s in range(NS):
                    eng = nc.vector if s < 2 else nc.gpsimd
                    eng.tensor_tensor(out=xcq8_sb[:, s, sl], in0=xq_sb[:, s, sl],
                                      in1=mub, op=ALU.subtract)
                for j in range(NS):
                    ps = pap.tile([P, 512], f32, tag="pj", name=f"q{tch}_{j}")
                    for pr in range(2):
                        nc.tensor.matmul(ps, wq_sb[:, 2 * pr:2 * pr + 2, j * P:(j + 1) * P],
                                         xcq8_sb[:, 2 * pr:2 * pr + 2, sl],
                                         start=(pr == 0), stop=(pr == 1), perf_mode=DR)
                    nc.vector.tensor_tensor(out=qT_sb[:, j, sl], in0=ps,
                                            in1=rqb, op=ALU.mult)

            # DMA issue order: chunk-0 x first (unblocks the first stats
            # matmuls), then K/V weights, then the rest -- nothing big queues
            # ahead of the critical path
            k_chunk(0, pre=lambda: (
                nc.sync.dma_start(out=wk_sb, in_=wk8.ap()),
                nc.sync.dma_start(out=wv_sb, in_=wv8.ap()),
            ))
            k_chunk(1, pre=lambda: (
                [nc.sync.dma_start(out=xq_sb[:, s],
                                   in_=xq.ap()[s * P:(s + 1) * P, :].bitcast(f32r))
                 for s in range(NS)],
                nc.sync.dma_start(out=wq_sb, in_=wq8.ap()),
            ))
            q_chunk(0)
            k_chunk(2, pre=lambda: (
                [nc.sync.dma_start(out=mask_sb[:, sl_], in_=masks.ap()[:, sl_])
                 for sl_ in range(NSLOT)],
                nc.sync.dma_start(out=id_sb, in_=ident.ap()),
            ))
            q_chunk(1)
            k_chunk(3)

        # deferred weight loads (needed from phase C on) -- issued here so the
        # phase-A x stream isn't queued behind 9MB of weight DMA
        nc.sync.dma_start(out=wo_sb, in_=wo8.ap())
        nc.sync.dma_start(out=w1_sb, in_=w18.ap())
        nc.sync.dma_start(out=w2_sb, in_=w28.ap())

        # ============ Phase B: attention ============
        with tc.tile_pool(name="pP", bufs=4) as pp, \
             tc.tile_pool(name="pNum", bufs=16) as pnum, \
             tc.tile_pool(name="pDen", bufs=2) as pden, \
             tc.tile_pool(name="pRcp", bufs=2) as prcp, \
             tc.tile_pool(name="pRsrc", bufs=8) as prsrc, \
             tc.tile_pool(name="pRrep", bufs=4) as prrep, \
             tc.tile_pool(name="pSps", bufs=2, space="PSUM") as pbp, \
             tc.tile_pool(name="pAVps", bufs=2, space="PSUM") as pbo:
            # software-pipeline AV one step behind S/exp ACROSS slot and hp
            # boundaries so the tensor queue never drains at a boundary
            pending = None       # () -> None, emits the last issued ktp's AV
            deferred = []        # post-AV actions for slots whose AV is pending

            def flush():
                nonlocal pending
                if pending is not None:
                    pending()
                    pending = None
                while deferred:
                    deferred.pop(0)()

            for hp in range(NS):
                den_row = pden.tile([1, 2 * NSLOT * QC], f32, tag="den", name=f"den{hp}")
                nums = {}
                for slot in range(NSLOT):
                    nkt = EXTS[slot] // P
                    qsl = slice(slot * QC, (slot + 1) * QC)
                    po = [pbo.tile([66, QC], f32, tag=f"av{hi}", name=f"av{hp}_{slot}_{hi}")
                          for hi in range(2)]

                    def emit_av(ktp, p_tile, po=po, nkt=nkt, hp=hp):
                        for hi in range(2):
                            h_loc = 2 * hp + hi
                            nc.tensor.matmul(
                                po[hi],
                                vaug_sb[:, 2 * ktp:2 * ktp + 2, h_loc, :],
                                p_tile[:, hi],
                                start=(ktp == 0), stop=(ktp == nkt // 2 - 1),
                                perf_mode=DR)

                    def slot_fin(po=po, hp=hp, slot=slot, den_row=den_row, nums=nums):
                        qsl_ = slice(slot * QC, (slot + 1) * QC)
                        for hi in range(2):
                            num = pnum.tile([64, QC], bf16, tag="num",
                                            name=f"n{hp}_{slot}_{hi}")
                            nc.vector.tensor_copy(out=num, in_=po[hi][0:64, :])
                            nums[(slot, hi)] = num
                            off = (2 * slot + hi) * QC
                            if hp == NS - 1:
                                # last hp: normalize per-slot immediately via
                                # scalar Ln/Exp (1/(WS*den) = exp(-ln den - ln WS))
                                # -- the batched DRAM-roundtrip wave would sit
                                # exposed in front of Wo
                                rr = prsrc.tile([1, QC], f32, tag="rr1",
                                                name=f"rr1_{slot}_{hi}")
                                nc.scalar.activation(out=rr, in_=po[hi][64:65, :],
                                                     func=AF.Ln)
                                nc.scalar.activation(out=rr, in_=rr, func=AF.Exp,
                                                     scale=-1.0, bias=nls_sb)
                                rrep = prrep.tile([64, QC], f32, tag="rrep",
                                                  name=f"rrL{slot}_{hi}")
                                nc.gpsimd.partition_broadcast(rrep, rr)
                                eng = nc.gpsimd if hi == 0 else nc.vector
                                eng.tensor_tensor(
                                    out=attnT_sb[hi * 64:(hi + 1) * 64, NS - 1, qsl_],
                                    in0=num, in1=rrep, op=ALU.mult)
                            else:
                                nc.vector.tensor_scalar_mul(
                                    out=den_row[:, off:off + QC], in0=po[hi][64:65, :],
                                    scalar1=WS)

                    for ktp in range(nkt // 2):
                        sp = pbp.tile([P, 2, 2, QC], f32, tag="spair",
                                      name=f"s{hp}_{slot}_{ktp}")
                        for i in range(2):
                            kt = 2 * ktp + i
                            ksl = slice(kt * P, (kt + 1) * P)
                            msk = kt >= nkt - 4
                            nc.tensor.matmul(sp[:, 0, i, :], kT_sb[0:64, hp, ksl],
                                             qT_sb[0:64, hp, qsl], start=True,
                                             stop=not msk)
                            nc.tensor.matmul(sp[:, 1, i, :], kT_sb[64:128, hp, ksl],
                                             qT_sb[64:128, hp, qsl], start=True,
                                             stop=not msk)
                            if msk:
                                # accumulate the additive mask (doubled: both
                                # heads) via one identity matmul -- keeps the
                                # exp dependency chain off DVE
                                m = mask_sb[:, slot, kt - (nkt - 4)]
                                nc.tensor.matmul(
                                    sp.rearrange("p h i q -> p i h q")[:, i],
                                    id_sb, m, start=False, stop=True)
                        pt = pp.tile([P, 2, 2, QC], fp8, tag="p",
                                     name=f"p{hp}_{slot}_{ktp}")
                        nc.scalar.activation(out=pt, in_=sp, func=AF.Exp, bias=lnw_col)
                        flush()
                        pending = (lambda ktp=ktp, pt=pt, emit=emit_av: emit(ktp, pt))
                    deferred.append(slot_fin)

                def hp_fin(hp=hp, den_row=den_row, nums=nums):
                    if hp == NS - 1:
                        return
                    # batched denominator reciprocal via DRAM partition-spread
                    nc.sync.dma_start(out=scr_den.ap()[hp:hp + 1, :], in_=den_row)
                    den8 = prcp.tile([2 * NSLOT, QC], f32, tag="den8", name=f"d8{hp}")
                    nc.sync.dma_start(
                        out=den8,
                        in_=scr_den.ap()[hp:hp + 1, :].rearrange("a (o q) -> (a o) q", o=2 * NSLOT))
                    rcp8 = prcp.tile([2 * NSLOT, QC], f32, tag="rcp8", name=f"r8{hp}")
                    nc.vector.reciprocal(out=rcp8, in_=den8)
                    nc.sync.dma_start(
                        out=scr_rcp.ap()[hp:hp + 1, :].rearrange("a (o q) -> (a o) q", o=2 * NSLOT),
                        in_=rcp8)
                    for slot in range(NSLOT):
                        qsl = slice(slot * QC, (slot + 1) * QC)
                        for hi in range(2):
                            off = (2 * slot + hi) * QC
                            rsrc = prsrc.tile([1, QC], f32, tag="rsrc",
                                              name=f"rs{hp}_{slot}_{hi}")
                            nc.sync.dma_start(out=rsrc, in_=scr_rcp.ap()[hp:hp + 1, off:off + QC])
                            rrep = prrep.tile([64, QC], f32, tag="rrep",
                                              name=f"rr{hp}_{slot}_{hi}")
                            nc.gpsimd.partition_broadcast(rrep, rsrc)
                            eng = nc.gpsimd if hi == 0 else nc.vector
                            eng.tensor_tensor(
                                out=attnT_sb[hi * 64:(hi + 1) * 64, hp, qsl],
                                in0=nums[(slot, hi)], in1=rrep, op=ALU.mult)
                deferred.append(hp_fin)
            flush()

        # ============ Phase C: Wo + residual + LN2 ============
        pcd = top.enter_context(tc.tile_pool(name="pCD", bufs=1))
        xnew_sb = pcd.tile([P, NS, TQ], f32r, tag="xnew")
        xc2_sb = pcd.tile([P, NS, TQ], fp8, tag="xc2")
        r2b_sb = pcd.tile([P, TQ], f32, tag="r2b")
        with tc.tile_pool(name="pC2", bufs=4) as pcc, \
             tc.tile_pool(name="pCrow", bufs=2) as pcr, \
             tc.tile_pool(name="pCps", bufs=4, space="PSUM") as pcp, \
             tc.tile_pool(name="pCst", bufs=2, space="PSUM") as pcs:
            for tch in range(TQ // 512):
                sl = slice(tch * 512, (tch + 1) * 512)
                wops = []
                for j in range(NS):
                    ps = pcp.tile([P, 512], f32, tag="wo", name=f"wo{tch}_{j}")
                    nc.tensor.matmul(ps, wo_sb[:, 0:2, j * P:(j + 1) * P],
                                     attnT_sb[:, 0:2, sl],
                                     start=True, stop=False, perf_mode=DR)
                    wops.append(ps)
                for j in range(NS):
                    ps = wops[j]
                    nc.tensor.matmul(ps, wo_sb[:, 2:4, j * P:(j + 1) * P],
                                     attnT_sb[:, 2:4, sl],
                                     start=False, stop=True, perf_mode=DR)
                    nc.vector.tensor_tensor(out=xnew_sb[:, j, sl], in0=ps,
                                            in1=xq_sb[:, j, sl], op=ALU.add)
            for tch in range(TQ // 512):
                sl = slice(tch * 512, (tch + 1) * 512)
                ps_x = pcs.tile([1, 512], f32, tag="st_x", name=f"m2_{tch}")
                ps_2 = pcs.tile([1, 512], f32, tag="st_2", name=f"v2_{tch}")
                for s in range(NS):
                    nc.tensor.matmul(ps_x, ones_r, xnew_sb[:, s, sl],
                                     start=(s == 0), stop=(s == NS - 1))
                for s in range(NS):
                    x2 = pcc.tile([P, 512], f32r, tag="x2n", name=f"x2n{tch}_{s}")
                    nc.scalar.activation(out=x2, in_=xnew_sb[:, s, sl], func=AF.Square)
                    nc.tensor.matmul(ps_2, ones_r, x2, start=(s == 0), stop=(s == NS - 1))
                mu2 = pcr.tile([1, 512], f32, tag="mu2", name=f"mu2_{tch}")
                rstd2 = pcr.tile([1, 512], f32, tag="rstd2", name=f"rstd2_{tch}")
                t2 = pcr.tile([1, 512], f32, tag="t2", name=f"t2_{tch}")
                nc.vector.tensor_scalar_mul(out=mu2, in0=ps_x, scalar1=1.0 / C)
                nc.vector.tensor_scalar_mul(out=rstd2, in0=ps_2, scalar1=1.0 / C)
                nc.vector.tensor_tensor(out=t2, in0=mu2, in1=mu2, op=ALU.mult)
                nc.vector.tensor_tensor(out=rstd2, in0=rstd2, in1=t2, op=ALU.subtract)
                nc.scalar.activation(out=rstd2, in_=rstd2, func=AF.Ln, bias=eps_sb)
                # rstd2/WS^2 (FF1 and FF2 weights are both 64x-scaled)
                nc.scalar.activation(out=rstd2, in_=rstd2, func=AF.Exp,
                                     scale=-0.5, bias=nls2_sb)
                mu2b = pcc.tile([P, 512], f32, tag="mu2b", name=f"mu2b{tch}")
                nc.gpsimd.partition_broadcast(mu2b, mu2)
                nc.gpsimd.partition_broadcast(r2b_sb[:, sl], rstd2)
                for s in range(NS):
                    nc.vector.tensor_tensor(out=xc2_sb[:, s, sl], in0=xnew_sb[:, s, sl],
                                            in1=mu2b, op=ALU.subtract)

        # ============ Phase D: FFN ============
        with tc.tile_pool(name="pD", bufs=2) as pd, \
             tc.tile_pool(name="pDy", bufs=3) as pdy, \
             tc.tile_pool(name="pDps", bufs=6, space="PSUM") as pdp:
            for tch in range(TQ // 512):
                sl = slice(tch * 512, (tch + 1) * 512)
                aT = pd.tile([P, F // P, 512], fp8, tag="aT", name=f"aT{tch}")
                for fj in range(F // P):
                    ps = pdp.tile([P, 512], f32, tag="ff", name=f"ff1_{tch}_{fj}")
                    for pr in range(2):
                        nc.tensor.matmul(ps, w1_sb[:, 2 * pr:2 * pr + 2, fj * P:(fj + 1) * P],
                                         xc2_sb[:, 2 * pr:2 * pr + 2, sl],
                                         start=(pr == 0), stop=(pr == 1), perf_mode=DR)
                    nc.scalar.activation(out=aT[:, fj], in_=ps, func=AF.Relu)
                for j in range(NS):
                    ps = pdp.tile([P, 512], f32, tag="ff", name=f"ff2_{tch}_{j}")
                    for pr in range(F // P // 2):
                        nc.tensor.matmul(ps, w2_sb[:, 2 * pr:2 * pr + 2, j * P:(j + 1) * P],
                                         aT[:, 2 * pr:2 * pr + 2, :],
                                         start=(pr == 0), stop=(pr == F // P // 2 - 1),
                                         perf_mode=DR)
                    yt = pdy.tile([P, 512], f32, tag="yt", name=f"y{tch}_{j}")
                    nc.vector.tensor_tensor(out=yt, in0=ps, in1=r2b_sb[:, sl], op=ALU.mult)
                    nc.vector.tensor_tensor(out=yt, in0=yt, in1=xnew_sb[:, j, sl], op=ALU.add)
                    nc.sync.dma_start(out=yT.ap()[j * P:(j + 1) * P, sl], in_=yt)

    with tile.TileContext(nc) as tc, contextlib.ExitStack() as top:
        _body(tc, top)
    nc.finalize()
    return nc


_prog = None


def _get_program():
    global _prog
    if _prog is None:
        _prog = _build_program()
    return _prog


def _q8(a):
    return np.ascontiguousarray(
        np.asarray(a, np.float32).astype(ml_dtypes.float8_e4m3fn).view(np.uint8))


def _host_prep(x, Wq, Wk, Wv, Wo, bo, g1, b1, g2, b2, W_ff1, b_ff1, W_ff2, b_ff2):
    x = np.asarray(x, np.float32)
    for nm, v in (("bo", bo), ("b1", b1), ("b2", b2), ("b_ff1", b_ff1), ("b_ff2", b_ff2)):
        if not np.allclose(np.asarray(v), 0.0):
            raise NotImplementedError(f"nonzero bias {nm} not supported")
    g1 = np.asarray(g1, np.float32)
    g2 = np.asarray(g2, np.float32)
    scale = np.float32(np.float64(C) ** -0.5)

    def pack(wT):   # [C_in, M] -> [128, NS_in, M]
        ki = wT.shape[0]
        return wT.reshape(ki // P, P, -1).transpose(1, 0, 2)

    wq8 = _q8(pack((np.asarray(Wq) * (g1 * scale)[None, :]).T * WS))
    wk8 = _q8(pack((np.asarray(Wk) * g1[None, :]).T * WS))
    wv8 = _q8(pack((np.asarray(Wv) * g1[None, :]).T * WS))
    wo8 = _q8(pack(np.asarray(Wo).T * WS))
    w18 = _q8(pack((np.asarray(W_ff1) * g2[None, :]).T * WS))
    w28 = _q8(pack(np.asarray(W_ff2).T * WS))
    shared = dict(
        wq8=wq8, wk8=wk8, wv8=wv8, wo8=wo8, w18=w18, w28=w28,
        cst=np.ones((P, P), np.float32),
        ident=_qb(np.eye(P, dtype=np.float32)),
    )
    in_maps = []
    for core in range(8):
        b, g = core // 2, core % 2
        chunks = [2 * i + g for i in range(NSLOT)]
        qrows = np.concatenate([np.arange(QC * ch, QC * (ch + 1)) for ch in chunks])
        m = np.zeros((P, NSLOT, 4, 2 * QC), np.float32)
        for i, ch in enumerate(chunks):
            for kr in range(4):
                kt = (EXTS[i] // P - 4) + kr
                k_abs = P * kt + np.arange(P)[:, None]
                q_abs = QC * ch + np.arange(QC)[None, :]
                mm_ = np.where(k_abs <= q_abs, 0.0, -30000.0)
                m[:, i, kr, 0:QC] = mm_
                m[:, i, kr, QC:] = mm_
        in_maps.append(dict(
            shared,
            xkT=np.ascontiguousarray(x[b].T),
            xq=np.ascontiguousarray(x[b][qrows].T),
            masks=_qb(m),
        ))
    return in_maps


def kernel(**inputs):
    global _last_exec_time_ns, _last_results
    inputs = {k: np.asarray(v) for k, v in inputs.items()}
    in_maps = _host_prep(**inputs)
    nc = _get_program()
    trace = os.environ.get("KERNEL_TRACE", "0") == "1"
    res = run_bass_kernel_spmd(nc, in_maps, list(range(8)), trace=trace)
    _last_exec_time_ns = res.exec_time_ns
    _last_results = res
    out = np.empty((4, T, C), np.float32)
    for core in range(8):
        b, g = core // 2, core % 2
        yt = res.results[core]["yT"]
        for i in range(NSLOT):
            ch = 2 * i + g
            out[b, QC * ch:QC * (ch + 1), :] = yt[:, QC * i:QC * (i + 1)].T
    return out
